# revision 1
# baseline (speedup 1.0000x reference)
"""Cross-conditional GPT2 sparse attention block on 8 Trainium2 NeuronCores.

Sharding: core = (batch b in 0..3) x (head-group g in 0..1, 6 heads each).
Each core computes, for its (b, g):
  qT/kT = (Wq_g @ x_b^T + bq_g)  laid out [d_on_partitions, L]
  v     = x_b @ Wv_g^T + bv_g    natural layout [L, 384], interleaved with a
          ones column per head ([L, 6, 65]) so att@v also yields the softmax
          denominator for free.
  scores are computed *transposed* (sT[j, i]) so that softmax needs no
  transpose at all: exp on ACT, multiplicative 0/1 mask (host-built, bf16),
  att@v via lhsT=v (natural layout), denominator broadcast across partitions
  via a K=1 PE matmul, then the partial output projection with Wp[:, g]^T.
Host sums the two per-batch partials and adds bp.
"""

import sys

sys.path.insert(0, "/opt/trn_rl_repo")

from contextlib import ExitStack

import ml_dtypes
import numpy as np

import concourse.bacc as bacc
import concourse.bass as bass
import concourse.mybir as mybir
import concourse.tile as tile
from concourse.bass_utils import run_bass_kernel_spmd

# ---- problem constants (hardcoded per spec) ----
B = 4
T = 512
N = 8
C = 768
NHEAD = 12
L = 3 * T + 4 * N  # 1568
P = 128
G = C // 2  # 384 channels per head-group
NH = 6  # heads per core
D = 64  # head dim
ET = C // P  # 6 e-tiles (contraction of x @ W)
CT = G // P  # 3 c-tiles of the group's channels
NJT = (L + P - 1) // P  # 13 j tiles (12x128 + 32)
JPAD = NJT * P  # 1664
I_CHUNKS = [(0, 512), (512, 512), (1024, 512), (1536, 32)]
SCALE = 1.0 / 8.0  # 1/sqrt(64)

F32 = mybir.dt.float32
BF16 = mybir.dt.bfloat16
F16 = mybir.dt.float16

_NC = None  # cached compiled Bass program


def _jl(jt):
    return P if jt < NJT - 1 else L - (NJT - 1) * P  # 128 or 32


def _score_intervals(jt):
    """i-ranges (start, len) that can attend any column in j-tile jt.
    Derived from the cross-conditional mask block structure. The text-row
    strip [1536,1568) is merged into the preceding torso interval whenever
    the combined length fits one PSUM bank (<=512)."""
    if jt <= 3:
        j0 = jt * P
        iv = [(j0, 512 - j0), (512 + j0, 512 - j0), (1024 + j0, 512 - j0), (1536, 32)]
    elif jt <= 11:
        f0 = (jt % 4) * P
        iv = [(512 + f0, 512 - f0), (1024 + f0, 512 - f0), (1536, 32)]
    else:
        iv = [(512, 512), (1024, 512), (1536, 32)]
    if len(iv) >= 2 and iv[-2][0] + iv[-2][1] == 1536 and iv[-2][1] + 32 <= 512:
        iv = iv[:-2] + [(iv[-2][0], iv[-2][1] + 32)]
    return iv


def _ich_of(a):
    return 3 if a == 1536 else a // 512


_ATTV_LAST = {0: 3, 1: NJT - 1, 2: NJT - 1, 3: NJT - 1}  # last jt per ich

# (group) -> per-jt score interval (a, ln) and mask spec.
# g0 = upper rows (i 0..512), jts 0..3; g1 = lower rows; g2 = torso+text rows.
def _grp_interval(g, jt):
    j0 = jt * P
    f0 = (jt % 4) * P if jt <= 11 else 0
    if g == 0:
        return (j0, 512 - j0) if jt <= 3 else None
    if g == 1:
        s = j0 if jt <= 3 else f0
        return (512 + s, 512 - s)
    s = j0 if jt <= 3 else f0
    return (1024 + s, 544 - s)


# mask kind per (group, jt): 'T1' | 'T2' | 'TXT' | None
def _grp_mask(g, jt):
    if jt == 12:
        return "TXT" if g in (1, 2) else None
    if g == 0:
        return "T1"
    if g == 1:
        return "T1" if jt <= 3 else "T2"
    return "T1" if jt <= 7 else "T2"


_GRP_ITS = {0: range(0, 4), 1: range(4, 8), 2: range(8, 13)}



def _build_program():
    nc = bacc.Bacc("TRN2", target_bir_lowering=False, debug=False)

    xT_d = nc.dram_tensor("xT", [C, L], F16, kind="ExternalInput")
    wq_d = nc.dram_tensor("wqT", [C, G], F16, kind="ExternalInput")
    wk_d = nc.dram_tensor("wkT", [C, G], F16, kind="ExternalInput")
    wv_d = nc.dram_tensor("wvT", [C, G], F16, kind="ExternalInput")
    wp_d = nc.dram_tensor("wpT", [G, C], F16, kind="ExternalInput")
    bq_d = nc.dram_tensor("bqP", [P, CT], F32, kind="ExternalInput")
    bk_d = nc.dram_tensor("bkP", [P, CT], F32, kind="ExternalInput")
    bv_d = nc.dram_tensor("bvB", [P, G], F32, kind="ExternalInput")
    maskd_d = nc.dram_tensor("maskD", [P, 2, P], F16, kind="ExternalInput")
    maskt_d = nc.dram_tensor("maskTxt", [32, 1024], F16, kind="ExternalInput")
    out_d = nc.dram_tensor("out_part", [L, C], F32, kind="ExternalOutput")

    with tile.TileContext(nc) as tc, ExitStack() as big:
        persist = big.enter_context(tc.tile_pool(name="persist", bufs=1))

        # persistent SBUF tensors
        qT = persist.tile([P, CT, L], F16, name="qT")
        kT = persist.tile([P, CT, L], F16, name="kT")
        v_ones = persist.tile([P, NJT, NH, D + 1], F16, name="v_ones")
        maskD = persist.tile([P, 2, P], F16, name="maskD_sb")
        maskTx = persist.tile([32, 1024], F16, name="maskTx_sb")
        yT = persist.tile([P, CT, L], F16, name="yT")
        wp_sb = persist.tile([P, CT, C], F16, name="wp_sb")
        ones64 = persist.tile([1, D], F16, name="ones64")
        bv_sb = persist.tile([P, G], F32, name="bv_sb")

        nc.sync.dma_start(maskD[:], maskd_d[:])
        nc.sync.dma_start(maskTx[:], maskt_d[:])
        nc.sync.dma_start(wp_sb[:], wp_d.rearrange("(ct p) n -> p ct n", p=P))
        nc.sync.dma_start(bv_sb[:], bv_d[:])
        nc.gpsimd.memset(ones64[:], 1.0)
        nc.gpsimd.memset(v_ones[:], 1.0)

        # ---------- Phase A: projections ----------
        with (
            tc.tile_pool(name="phA", bufs=1) as phA,
            tc.tile_pool(name="psA", bufs=2, space="PSUM") as psA,
        ):
            xT = phA.tile([P, ET, L], F16, name="xT_sb")
            wq_sb = phA.tile([P, ET, G], F16, name="wq_sb")
            wk_sb = phA.tile([P, ET, G], F16, name="wk_sb")
            wv_sb = phA.tile([P, ET, G], F16, name="wv_sb")
            bq_sb = phA.tile([P, CT], F32, name="bq_sb")
            bk_sb = phA.tile([P, CT], F32, name="bk_sb")

            nc.sync.dma_start(xT[:], xT_d.rearrange("(et p) i -> p et i", p=P))
            nc.sync.dma_start(wq_sb[:], wq_d.rearrange("(et p) m -> p et m", p=P))
            nc.sync.dma_start(wk_sb[:], wk_d.rearrange("(et p) m -> p et m", p=P))
            nc.sync.dma_start(wv_sb[:], wv_d.rearrange("(et p) m -> p et m", p=P))
            nc.sync.dma_start(bq_sb[:], bq_d[:])
            nc.sync.dma_start(bk_sb[:], bk_d[:])

            # qT / kT: out[c_tile, i] accumulated over e tiles
            for dst, w_sb, b_sb in ((qT, wq_sb, bq_sb), (kT, wk_sb, bk_sb)):
                for ct in range(CT):
                    for i0, ilen in I_CHUNKS:
                        ps = psA.tile([P, 512], F32, name="ps_qk", tag="ps_qk")
                        for et in range(ET):
                            nc.tensor.matmul(
                                ps[:, :ilen],
                                w_sb[:, et, ct * P : (ct + 1) * P],
                                xT[:, et, i0 : i0 + ilen],
                                start=(et == 0),
                                stop=(et == ET - 1),
                            )
                        nc.vector.tensor_scalar(
                            dst[:, ct, i0 : i0 + ilen],
                            ps[:, :ilen],
                            b_sb[:, ct : ct + 1],
                            None,
                            mybir.AluOpType.add,
                        )

            # v natural layout [i, 384] + bias, into the 65-strided bf16 buffer
            for it in range(NJT):
                il = _jl(it)
                ps = psA.tile([P, G], F32, name="ps_v", tag="ps_v")
                for et in range(ET):
                    nc.tensor.matmul(
                        ps[:il, :],
                        xT[:, et, it * P : it * P + il],
                        wv_sb[:, et, :],
                        start=(et == 0),
                        stop=(et == ET - 1),
                    )
                nc.vector.tensor_tensor(
                    v_ones[:il, it, :, 0:D],
                    ps[:il, :].rearrange("p (h d) -> p h d", h=NH),
                    bv_sb[:il, :].rearrange("p (h d) -> p h d", h=NH),
                    mybir.AluOpType.add,
                )

        # ---------- Phase B+C: attention by row-group, proj interleaved ----------
        with (
            tc.tile_pool(name="phB", bufs=1) as phB,
            tc.tile_pool(name="phC", bufs=3) as phC,
            tc.tile_pool(name="psS", bufs=3, space="PSUM") as psS,
            tc.tile_pool(name="psY", bufs=5, space="PSUM") as psY,
        ):
            for g in range(3):
                jts = [jt for jt in range(NJT) if _grp_interval(g, jt) is not None]
                for h in range(NH):
                    pof = D * (h % 2)
                    ct = h // 2
                    ps_y = {}
                    started = set()
                    for jt in jts:
                        jl = _jl(jt)
                        a, ln = _grp_interval(g, jt)
                        chunks = [(a, min(ln, 512))]
                        if ln > 512:
                            chunks.append((a + 512, ln - 512))
                        for ca, cl in chunks:
                            ps_s = psS.tile([P, 512], F32, name="ps_s", tag="ps_s")
                            nc.tensor.matmul(
                                ps_s[:jl, :cl],
                                kT[pof : pof + D, ct, jt * P : jt * P + jl],
                                qT[pof : pof + D, ct, ca : ca + cl],
                                start=True,
                                stop=True,
                            )
                            pt = phB.tile([P, 512], F16, name="pT", tag="pT", bufs=16)
                            nc.scalar.activation(
                                pt[:jl, :cl],
                                ps_s[:jl, :cl],
                                mybir.ActivationFunctionType.Exp,
                                bias=0.0,
                                scale=SCALE,
                            )
                            mk = _grp_mask(g, jt)
                            if ca == a and mk in ("T1", "T2"):
                                nc.vector.tensor_tensor(
                                    pt[:jl, 0:P],
                                    pt[:jl, 0:P],
                                    maskD[:jl, 0 if mk == "T1" else 1, :],
                                    mybir.AluOpType.mult,
                                )
                            elif ca == a and mk == "TXT":
                                m0 = a - 512
                                nc.vector.tensor_tensor(
                                    pt[:jl, :cl],
                                    pt[:jl, :cl],
                                    maskTx[:jl, m0 : m0 + cl],
                                    mybir.AluOpType.mult,
                                )
                            parts = [(ca, cl, 0)]
                            if ca < 1536 < ca + cl:
                                parts = [
                                    (ca, 1536 - ca, 0),
                                    (1536, ca + cl - 1536, 1536 - ca),
                                ]
                            for pa, pl, poff in parts:
                                ich = _ich_of(pa)
                                off = pa - (0, 512, 1024, 1536)[ich]
                                if ich not in ps_y:
                                    ps_y[ich] = psY.tile(
                                        [D + 1, 512], F32, name=f"ps_y{ich}", tag="ps_y"
                                    )
                                nc.tensor.matmul(
                                    ps_y[ich][:, off : off + pl],
                                    v_ones[:jl, jt, h, :],
                                    pt[:jl, poff : poff + pl],
                                    start=ich not in started,
                                    stop=(jt == jts[-1]),
                                    skip_group_check=True,
                                )
                                started.add(ich)

                    for ich, psy in ps_y.items():
                        i0, ilen = I_CHUNKS[ich]
                        den = phB.tile([1, 512], F16, name="den", tag="den", bufs=4)
                        nc.vector.tensor_copy(den[0:1, :ilen], psy[D : D + 1, :ilen])
                        ps_bc = psS.tile([D, 512], F32, name="ps_bc", tag="ps_s")
                        nc.tensor.matmul(
                            ps_bc[:, :ilen],
                            ones64[0:1, :],
                            den[0:1, :ilen],
                            start=True,
                            stop=True,
                        )
                        rc = phB.tile([D, 512], F32, name="rc", tag="rc", bufs=4)
                        nc.vector.reciprocal_approx_fast(
                            out=rc[:, :ilen], in_=ps_bc[:, :ilen]
                        )
                        nc.vector.tensor_tensor(
                            yT[pof : pof + D, ct, i0 : i0 + ilen],
                            psy[0:D, :ilen],
                            rc[:, :ilen],
                            mybir.AluOpType.mult,
                        )

                # output projection for this group's row tiles
                for it in _GRP_ITS[g]:
                    il = _jl(it)
                    o_sb = phC.tile([P, C], F32, name="o_sb", tag="o_sb")
                    for nch in range(2):
                        ps_o = psS.tile([P, 512], F32, name="ps_o", tag="ps_s")
                        for kt in range(CT):
                            nc.tensor.matmul(
                                ps_o[:il, :384],
                                yT[:, kt, it * P : it * P + il],
                                wp_sb[:, kt, nch * 384 : (nch + 1) * 384],
                                start=(kt == 0),
                                stop=(kt == CT - 1),
                                skip_group_check=True,
                            )
                        nc.any.tensor_copy(
                            o_sb[:il, nch * 384 : (nch + 1) * 384], ps_o[:il, :384]
                        )
                    nc.sync.dma_start(out_d[it * P : it * P + il, :], o_sb[:il, :])

    nc.compile()
    return nc


def _build_mask_np(seg_starts, seg_ends):
    """True = masked. Mirrors reference._build_mask in numpy."""
    ML = 3 * T
    tril = np.tril(np.ones((T, T), dtype=bool))
    sl = np.tril(np.ones((T, T), dtype=bool), -1)
    m = np.zeros((L, L), dtype=bool)
    m[:ML, :ML] = True
    m[0:T, 0:T] = ~tril
    m[T : 2 * T, 0:T] = ~tril
    m[T : 2 * T, T : 2 * T] = ~sl
    m[T : 2 * T, 2 * T : 3 * T] = ~sl
    m[2 * T : 3 * T, 0:T] = ~tril
    m[2 * T : 3 * T, T : 2 * T] = ~tril
    m[2 * T : 3 * T, 2 * T : 3 * T] = ~sl
    m[:ML, ML:] = True
    frames = np.arange(T)[None, :, None]
    allowed = (frames >= seg_starts[:, None, :]) & (frames < seg_ends[:, None, :])
    mask = np.broadcast_to(m[None], (B, L, L)).copy()
    for row0, col_blocks in ((T, (0, 2, 3)), (2 * T, (1, 2, 3))):
        for j in col_blocks:
            c0 = ML + j * N
            mask[:, row0 : row0 + T, c0 : c0 + N] &= ~allowed
    return mask


def get_nc():
    global _NC
    if _NC is None:
        _NC = _build_program()
    return _NC


def make_in_maps(x, Wq, bq, Wk, bk, Wv, bv, Wp, bp, seg_starts, seg_ends):
    mask = _build_mask_np(np.asarray(seg_starts), np.asarray(seg_ends))
    r = np.arange(P)
    maskD = np.empty((P, 2, P), dtype=np.float16)
    maskD[:, 0, :] = (r[:, None] <= r[None, :]).astype(np.float16)  # tril.T
    maskD[:, 1, :] = (r[:, None] < r[None, :]).astype(np.float16)  # strict
    in_maps = []
    for core in range(8):
        b, g = core // 2, core % 2
        gs = slice(g * G, (g + 1) * G)
        allowT = ~mask[b].T  # [j, i]
        maskTx = np.ascontiguousarray(
            allowT[1536:1568, 512:1536].astype(np.float16)
        )
        in_maps.append(
            {
                "xT": np.ascontiguousarray(x[b].T).astype(np.float16),
                "wqT": np.ascontiguousarray(Wq[gs, :].T).astype(np.float16),
                "wkT": np.ascontiguousarray(Wk[gs, :].T).astype(np.float16),
                "wvT": np.ascontiguousarray(Wv[gs, :].T).astype(np.float16),
                "wpT": np.ascontiguousarray(Wp[:, gs].T).astype(np.float16),
                "bqP": np.ascontiguousarray(bq[gs].reshape(CT, P).T),
                "bkP": np.ascontiguousarray(bk[gs].reshape(CT, P).T),
                "bvB": np.broadcast_to(bv[gs], (P, G)).copy(),
                "maskD": maskD,
                "maskTxt": maskTx,
            }
        )
    return in_maps


def kernel(x, Wq, bq, Wk, bk, Wv, bv, Wp, bp, seg_starts, seg_ends, T_motion=None,
           N=None, _trace=False, **_unused):
    x = np.asarray(x, np.float32)
    args = [np.asarray(a, np.float32) for a in (Wq, bq, Wk, bk, Wv, bv, Wp, bp)]
    Wq, bq, Wk, bk, Wv, bv, Wp, bp = args
    nc = get_nc()
    in_maps = make_in_maps(x, Wq, bq, Wk, bk, Wv, bv, Wp, bp, seg_starts, seg_ends)
    res = run_bass_kernel_spmd(nc, in_maps, core_ids=list(range(8)), trace=_trace)
    parts = [r["out_part"] for r in res.results]
    y = np.empty((B, L, C), np.float32)
    for b in range(B):
        y[b] = parts[2 * b] + parts[2 * b + 1] + bp
    if _trace:
        kernel.last_results = res
    return y



# revision 8
# speedup vs baseline: 1.1331x; 1.1331x over previous
"""Cross-conditional GPT2 sparse attention block on 8 Trainium2 NeuronCores.

Sharding: core = (batch b in 0..3) x (head-group g in 0..1, 6 heads each).
Each core computes, for its (b, g):
  qT/kT = (Wq_g @ x_b^T + bq_g)  laid out [d_on_partitions, L]
  v     = x_b @ Wv_g^T + bv_g    natural layout [L, 384], interleaved with a
          ones column per head ([L, 6, 65]) so att@v also yields the softmax
          denominator for free.
  scores are computed *transposed* (sT[j, i]) so that softmax needs no
  transpose at all: exp on ACT, multiplicative 0/1 mask (host-built, f16),
  att@v via lhsT=v (natural layout), denominator broadcast across partitions
  via a K=1 PE matmul, then the partial output projection with Wp[:, g]^T.
Host sums the two per-batch partials and adds bp.

v2 performance structure (TRN2 PE p-states: the PE only reaches 2.4 GHz
after ~3us of gap-free execution; any stall drops it to 1.2 GHz):
  - all inputs are host-pre-swizzled so every DMA is contiguous per
    partition (128 fat descriptors/tensor instead of ~per-row thin ones;
    descriptor issue rate, not bandwidth, dominated the old 22us lead-in)
  - phase A loops i-chunk-outer so compute starts once chunk 0 lands
  - the attention phase is software-pipelined: a flat work-queue of score
    chunks is emitted with LOOK items of lookahead, so the in-order PE
    queue always has an independent score matmul to run while the
    exp (ACT) -> mask (DVE) -> att@v chain of an earlier chunk drains.
    Output-projection chains are interleaved into the following group's
    stream as additional PE filler; softmax finalize is split in two
    stages (cast on Pool, then bcast/recip/mult) two slots apart.
"""

import sys

sys.path.insert(0, "/opt/trn_rl_repo")

from contextlib import ExitStack

import ml_dtypes
import numpy as np

import concourse.bacc as bacc
import concourse.bass as bass
import concourse.mybir as mybir
import concourse.tile as tile
from concourse.bass_utils import run_bass_kernel_spmd

# ---- problem constants (hardcoded per spec) ----
B = 4
T = 512
N = 8
C = 768
NHEAD = 12
L = 3 * T + 4 * N  # 1568
P = 128
G = C // 2  # 384 channels per head-group
NH = 6  # heads per core
D = 64  # head dim
ET = C // P  # 6 e-tiles (contraction of x @ W)
CT = G // P  # 3 c-tiles of the group's channels
NJT = (L + P - 1) // P  # 13 j tiles (12x128 + 32)
I_CHUNKS = [(0, 512), (512, 512), (1024, 512), (1536, 32)]
SCALE = 1.0 / 8.0  # 1/sqrt(64)
LOOK = 3  # attention pipeline lookahead (chunks in flight ahead of att@v)

F32 = mybir.dt.float32
BF16 = mybir.dt.bfloat16
F16 = mybir.dt.float16

_NC = None  # cached compiled Bass program


def _jl(jt):
    return P if jt < NJT - 1 else L - (NJT - 1) * P  # 128 or 32


def _ich_of(a):
    return 3 if a == 1536 else a // 512


# (group) -> per-jt score interval (a, ln).
# g0 = upper rows (i 0..512), jts 0..3; g1 = lower rows; g2 = torso+text rows.
def _grp_interval(g, jt):
    j0 = jt * P
    f0 = (jt % 4) * P if jt <= 11 else 0
    if g == 0:
        return (j0, 512 - j0) if jt <= 3 else None
    if g == 1:
        s = j0 if jt <= 3 else f0
        return (512 + s, 512 - s)
    s = j0 if jt <= 3 else f0
    return (1024 + s, 544 - s)


# mask kind per (group, jt): 'T1' | 'T2' | 'TXT' | None
def _grp_mask(g, jt):
    if jt == 12:
        return "TXT" if g in (1, 2) else None
    if g == 0:
        return "T1"
    if g == 1:
        return "T1" if jt <= 3 else "T2"
    return "T1" if jt <= 7 else "T2"


_GRP_ITS = {0: range(0, 4), 1: range(4, 8), 2: range(8, 13)}


def _grp_jts(g):
    return [jt for jt in range(NJT) if _grp_interval(g, jt) is not None]


def _chunks_of(g, jt):
    a, ln = _grp_interval(g, jt)
    out = [(a, min(ln, 512))]
    if ln > 512:
        out.append((a + 512, ln - 512))
    return out


def _attn_items():
    """Flat attention work queue. 'chunk' items carry one (g,h,jt,ca,cl)
    score chunk; 'oproj' items are output-projection chains for the
    PREVIOUS group, interleaved into this group's stream as PE filler."""
    items = []
    for g in range(3):
        chunk_items = []
        for h in range(NH):
            jts = _grp_jts(g)
            last_jt = jts[-1]
            head_chunks = []
            for jt in jts:
                for ca, cl in _chunks_of(g, jt):
                    head_chunks.append([g, h, jt, ca, cl, jt == last_jt, False])
            head_chunks[-1][6] = True  # final chunk of this head -> fin1
            chunk_items += [tuple(["chunk"] + hc) for hc in head_chunks]
        if g > 0:
            op = [
                ("oproj", g - 1, it, nch)
                for it in _GRP_ITS[g - 1]
                for nch in (0, 1)
            ]
            merged = []
            oi = 0
            for i, ci in enumerate(chunk_items):
                merged.append(ci)
                if oi < len(op) and i % 2 == 1:
                    merged.append(op[oi])
                    oi += 1
            merged += op[oi:]
            chunk_items = merged
        items += chunk_items
    items += [("oproj", 2, it, nch) for it in _GRP_ITS[2] for nch in (0, 1)]
    return items


def _build_program():
    nc = bacc.Bacc("TRN2", target_bir_lowering=False, debug=False)

    xa_d = nc.dram_tensor("xa", [P, 3, ET, 512], F16, kind="ExternalInput")
    xb_d = nc.dram_tensor("xb", [P, ET, 32], F16, kind="ExternalInput")
    wq_d = nc.dram_tensor("wqS", [P, ET, G], F16, kind="ExternalInput")
    wk_d = nc.dram_tensor("wkS", [P, ET, G], F16, kind="ExternalInput")
    wv_d = nc.dram_tensor("wvS", [P, ET, G], F16, kind="ExternalInput")
    wp_d = nc.dram_tensor("wpS", [P, CT, C], F16, kind="ExternalInput")
    bq_d = nc.dram_tensor("bqP", [P, CT], F32, kind="ExternalInput")
    bk_d = nc.dram_tensor("bkP", [P, CT], F32, kind="ExternalInput")
    bv_d = nc.dram_tensor("bvB", [P, G], F32, kind="ExternalInput")
    maskd_d = nc.dram_tensor("maskD", [P, 2, P], F16, kind="ExternalInput")
    maskt_d = nc.dram_tensor("maskTxt", [32, 1024], F16, kind="ExternalInput")
    out_d = nc.dram_tensor("out_part", [L, C], F32, kind="ExternalOutput")

    with tile.TileContext(nc) as tc, ExitStack() as big:
        persist = big.enter_context(tc.tile_pool(name="persist", bufs=1))

        # persistent SBUF tensors
        qT = persist.tile([P, CT, L], F16, name="qT")
        kT = persist.tile([P, CT, L], F16, name="kT")
        v_ones = persist.tile([P, NJT, NH, D + 1], F16, name="v_ones")
        maskD = persist.tile([P, 2, P], F16, name="maskD_sb")
        maskTx = persist.tile([32, 1024], F16, name="maskTx_sb")
        yT = persist.tile([P, CT, L], F16, name="yT")
        wp_sb = persist.tile([P, CT, C], F16, name="wp_sb")
        ones64 = persist.tile([1, D], F16, name="ones64")
        bv_sb = persist.tile([P, G], F32, name="bv_sb")

        nc.gpsimd.memset(ones64[:], 1.0)
        nc.gpsimd.memset(v_ones[:], 1.0)

        # ---------- Phase A: projections ----------
        with (
            tc.tile_pool(name="phA", bufs=1) as phA,
            tc.tile_pool(name="psA", bufs=2, space="PSUM") as psA,
        ):
            xTa = phA.tile([P, 3, ET, 512], F16, name="xTa_sb")
            xTb = phA.tile([P, ET, 32], F16, name="xTb_sb")
            wq_sb = phA.tile([P, ET, G], F16, name="wq_sb")
            wk_sb = phA.tile([P, ET, G], F16, name="wk_sb")
            wv_sb = phA.tile([P, ET, G], F16, name="wv_sb")
            bq_sb = phA.tile([P, CT], F32, name="bq_sb")
            bk_sb = phA.tile([P, CT], F32, name="bk_sb")

            # priority-ordered contiguous DMAs (128 descriptors each)
            nc.sync.dma_start(wq_sb[:], wq_d[:])
            nc.sync.dma_start(bq_sb[:], bq_d[:])
            nc.sync.dma_start(wk_sb[:], wk_d[:])
            nc.sync.dma_start(bk_sb[:], bk_d[:])
            for cchunk in range(3):
                nc.sync.dma_start(xTa[:, cchunk], xa_d[:, cchunk])
            nc.sync.dma_start(xTb[:], xb_d[:])
            nc.sync.dma_start(wv_sb[:], wv_d[:])
            nc.sync.dma_start(bv_sb[:], bv_d[:])
            nc.sync.dma_start(maskD[:], maskd_d[:])
            nc.sync.dma_start(maskTx[:], maskt_d[:])
            nc.sync.dma_start(wp_sb[:], wp_d[:])

            def xslice(ic, et, lo, ln):
                # xT columns [i0+lo : i0+lo+ln] of e-tile et for i-chunk ic
                if ic < 3:
                    return xTa[:, ic, et, lo : lo + ln]
                return xTb[:, et, lo : lo + ln]

            # qT / kT: out[c_tile, i] accumulated over e tiles; i-chunk
            # outer so compute starts as soon as x chunk 0 arrives.
            for ic, (i0, ilen) in enumerate(I_CHUNKS):
                for dst, w_sb, b_sb in ((qT, wq_sb, bq_sb), (kT, wk_sb, bk_sb)):
                    for ct in range(CT):
                        ps = psA.tile([P, 512], F32, name="ps_qk", tag="ps_qk")
                        for et in range(ET):
                            nc.tensor.matmul(
                                ps[:, :ilen],
                                w_sb[:, et, ct * P : (ct + 1) * P],
                                xslice(ic, et, 0, ilen),
                                start=(et == 0),
                                stop=(et == ET - 1),
                            )
                        nc.vector.tensor_scalar(
                            dst[:, ct, i0 : i0 + ilen],
                            ps[:, :ilen],
                            b_sb[:, ct : ct + 1],
                            None,
                            mybir.AluOpType.add,
                        )

            # v natural layout [i, 384] + bias, into the 65-strided f16 buffer
            for it in range(NJT):
                il = _jl(it)
                ic, lo = (it // 4, (it % 4) * P) if it < 12 else (3, 0)
                ps = psA.tile([P, G], F32, name="ps_v", tag="ps_v")
                for et in range(ET):
                    nc.tensor.matmul(
                        ps[:il, :],
                        xslice(ic, et, lo, il),
                        wv_sb[:, et, :],
                        start=(et == 0),
                        stop=(et == ET - 1),
                    )
                nc.vector.tensor_tensor(
                    v_ones[:il, it, :, 0:D],
                    ps[:il, :].rearrange("p (h d) -> p h d", h=NH),
                    bv_sb[:il, :].rearrange("p (h d) -> p h d", h=NH),
                    mybir.AluOpType.add,
                )

        # ---------- Phase B+C: pipelined attention + interleaved out-proj ----
        with (
            tc.tile_pool(name="phB", bufs=1) as phB,
            tc.tile_pool(name="phC", bufs=3) as phC,
            tc.tile_pool(name="psS", bufs=4, space="PSUM") as psS,
            tc.tile_pool(name="psY", bufs=2, space="PSUM") as psY,
            tc.tile_pool(name="psY3", bufs=2, space="PSUM") as psY3,
        ):
            items = _attn_items()
            nitems = len(items)
            # per-(g,h) attention state
            st = {}
            for g in range(3):
                for h in range(NH):
                    st[(g, h)] = {"psy": {}, "started": set(), "pt": {}}
            osb = {}
            deferred = []  # fin args emitted this slot
            fin_due = []  # [(slot_recorded, fin-args), ...]

            def emit_score(g, h, jt, ca, cl):
                s = st[(g, h)]
                jl = _jl(jt)
                a, _ = _grp_interval(g, jt)
                pof = D * (h % 2)
                ct = h // 2
                ps_s = psS.tile([P, 512], F32, name="ps_s", tag="ps_s")
                nc.tensor.matmul(
                    ps_s[:jl, :cl],
                    kT[pof : pof + D, ct, jt * P : jt * P + jl],
                    qT[pof : pof + D, ct, ca : ca + cl],
                    start=True,
                    stop=True,
                )
                pt = phB.tile([P, 512], F16, name="pT", tag="pT", bufs=14)
                nc.scalar.activation(
                    pt[:jl, :cl],
                    ps_s[:jl, :cl],
                    mybir.ActivationFunctionType.Exp,
                    bias=0.0,
                    scale=SCALE,
                )
                mk = _grp_mask(g, jt)
                if ca == a and mk in ("T1", "T2"):
                    nc.vector.tensor_tensor(
                        pt[:jl, 0:P],
                        pt[:jl, 0:P],
                        maskD[:jl, 0 if mk == "T1" else 1, :],
                        mybir.AluOpType.mult,
                    )
                elif ca == a and mk == "TXT":
                    m0 = ca - 512
                    nc.vector.tensor_tensor(
                        pt[:jl, :cl],
                        pt[:jl, :cl],
                        maskTx[:jl, m0 : m0 + cl],
                        mybir.AluOpType.mult,
                    )
                s["pt"][(jt, ca)] = pt

            def emit_attv(g, h, jt, ca, cl, stop):
                s = st[(g, h)]
                jl = _jl(jt)
                pt = s["pt"].pop((jt, ca))
                parts = [(ca, cl, 0)]
                if ca < 1536 < ca + cl:
                    parts = [
                        (ca, 1536 - ca, 0),
                        (1536, ca + cl - 1536, 1536 - ca),
                    ]
                for pa, pl, poff in parts:
                    ich = _ich_of(pa)
                    off = pa - (0, 512, 1024, 1536)[ich]
                    if ich not in s["psy"]:
                        pool = psY3 if ich == 3 else psY
                        s["psy"][ich] = pool.tile(
                            [D + 1, 512], F32, name=f"ps_y{ich}", tag=pool.name
                        )
                    nc.tensor.matmul(
                        s["psy"][ich][:, off : off + pl],
                        v_ones[:jl, jt, h, :],
                        pt[:jl, poff : poff + pl],
                        start=ich not in s["started"],
                        stop=stop,
                        skip_group_check=True,
                    )
                    s["started"].add(ich)

            def emit_fin1(g, h):
                """Stage 1 of softmax finalize: denominator row to SBUF f16
                (on Pool, which is otherwise idle)."""
                s = st[(g, h)]
                for ich, psy in s["psy"].items():
                    i0, ilen = I_CHUNKS[ich]
                    den = phB.tile([1, 512], F16, name="den", tag="den", bufs=4)
                    nc.vector.tensor_copy(den[0:1, :ilen], psy[D : D + 1, :ilen])
                    deferred.append((g, h, ich, den))

            def emit_fin2(g, h, ich, den):
                """Stage 2: broadcast den across partitions (K=1 PE matmul),
                reciprocal, multiply into yT."""
                s = st[(g, h)]
                pof = D * (h % 2)
                ct = h // 2
                psy = s["psy"][ich]
                i0, ilen = I_CHUNKS[ich]
                ps_bc = psS.tile([D, 512], F32, name="ps_bc", tag="ps_s")
                nc.tensor.matmul(
                    ps_bc[:, :ilen],
                    ones64[0:1, :],
                    den[0:1, :ilen],
                    start=True,
                    stop=True,
                )
                rc = phB.tile([D, 512], F32, name="rc", tag="rc", bufs=4)
                nc.vector.reciprocal_approx_fast(out=rc[:, :ilen], in_=ps_bc[:, :ilen])
                nc.vector.tensor_tensor(
                    yT[pof : pof + D, ct, i0 : i0 + ilen],
                    psy[0:D, :ilen],
                    rc[:, :ilen],
                    mybir.AluOpType.mult,
                )

            def drain_fins_for_group(g):
                keep = []
                for slot, args in fin_due:
                    if args[0] == g:
                        emit_fin2(*args)
                    else:
                        keep.append((slot, args))
                fin_due[:] = keep

            def emit_oproj(g, it, nch):
                drain_fins_for_group(g)
                il = _jl(it)
                if nch == 0:
                    osb[it] = phC.tile([P, C], F32, name="o_sb", tag="o_sb")
                o_sb = osb[it]
                ps_o = psS.tile([P, 512], F32, name="ps_o", tag="ps_s")
                for kt in range(CT):
                    nc.tensor.matmul(
                        ps_o[:il, :G],
                        yT[:, kt, it * P : it * P + il],
                        wp_sb[:, kt, nch * G : (nch + 1) * G],
                        start=(kt == 0),
                        stop=(kt == CT - 1),
                        skip_group_check=True,
                    )
                nc.any.tensor_copy(o_sb[:il, nch * G : (nch + 1) * G], ps_o[:il, :G])
                if nch == 1:
                    nc.sync.dma_start(out_d[it * P : it * P + il, :], o_sb[:il, :])
                    del osb[it]

            for i in range(nitems + LOOK):
                if i < nitems:
                    itm = items[i]
                    if itm[0] == "chunk":
                        emit_score(itm[1], itm[2], itm[3], itm[4], itm[5])
                if i >= LOOK:
                    # release fins that have aged >= 2 slots (their Pool
                    # cast has had time to finish; keeps the PE bcast from
                    # stalling right after the last att@v)
                    while fin_due and fin_due[0][0] <= i - 2:
                        _, args = fin_due.pop(0)
                        emit_fin2(*args)
                    itm = items[i - LOOK]
                    if itm[0] == "chunk":
                        _, g, h, jt, ca, cl, stop, final = itm
                        emit_attv(g, h, jt, ca, cl, stop)
                        if final:
                            emit_fin1(g, h)
                        while deferred:
                            fin_due.append((i, deferred.pop(0)))
                    else:
                        emit_oproj(itm[1], itm[2], itm[3])
            # drain any remaining fins
            while fin_due:
                _, args = fin_due.pop(0)
                emit_fin2(*args)

    nc.compile()
    return nc


def _build_mask_np(seg_starts, seg_ends):
    """True = masked. Mirrors reference._build_mask in numpy."""
    ML = 3 * T
    tril = np.tril(np.ones((T, T), dtype=bool))
    sl = np.tril(np.ones((T, T), dtype=bool), -1)
    m = np.zeros((L, L), dtype=bool)
    m[:ML, :ML] = True
    m[0:T, 0:T] = ~tril
    m[T : 2 * T, 0:T] = ~tril
    m[T : 2 * T, T : 2 * T] = ~sl
    m[T : 2 * T, 2 * T : 3 * T] = ~sl
    m[2 * T : 3 * T, 0:T] = ~tril
    m[2 * T : 3 * T, T : 2 * T] = ~tril
    m[2 * T : 3 * T, 2 * T : 3 * T] = ~sl
    m[:ML, ML:] = True
    frames = np.arange(T)[None, :, None]
    allowed = (frames >= seg_starts[:, None, :]) & (frames < seg_ends[:, None, :])
    mask = np.broadcast_to(m[None], (B, L, L)).copy()
    for row0, col_blocks in ((T, (0, 2, 3)), (2 * T, (1, 2, 3))):
        for j in col_blocks:
            c0 = ML + j * N
            mask[:, row0 : row0 + T, c0 : c0 + N] &= ~allowed
    return mask


def get_nc():
    global _NC
    if _NC is None:
        _NC = _build_program()
    return _NC


def _swz(mat, nt):
    """[nt*128, X] -> [128, nt, X] contiguous f16 (partition-major)."""
    x = np.asarray(mat)
    return np.ascontiguousarray(
        x.reshape(nt, P, x.shape[1]).transpose(1, 0, 2)
    ).astype(np.float16)


def make_in_maps(x, Wq, bq, Wk, bk, Wv, bv, Wp, bp, seg_starts, seg_ends):
    mask = _build_mask_np(np.asarray(seg_starts), np.asarray(seg_ends))
    r = np.arange(P)
    maskDh = np.empty((P, 2, P), dtype=np.float16)
    maskDh[:, 0, :] = (r[:, None] <= r[None, :]).astype(np.float16)  # tril.T
    maskDh[:, 1, :] = (r[:, None] < r[None, :]).astype(np.float16)  # strict
    in_maps = []
    for core in range(8):
        b, g = core // 2, core % 2
        gs = slice(g * G, (g + 1) * G)
        allowT = ~mask[b].T  # [j, i]
        maskTx = np.ascontiguousarray(
            allowT[1536:1568, 512:1536].astype(np.float16)
        )
        xsw = _swz(x[b].T, ET)  # [P, ET, L]
        xa = np.ascontiguousarray(
            np.stack([xsw[:, :, c * 512 : (c + 1) * 512] for c in range(3)], axis=1)
        )  # [P, 3, ET, 512]
        xb_ = np.ascontiguousarray(xsw[:, :, 1536:1568])  # [P, ET, 32]
        in_maps.append(
            {
                "xa": xa,
                "xb": xb_,
                "wqS": _swz(Wq[gs, :].T, ET),
                "wkS": _swz(Wk[gs, :].T, ET),
                "wvS": _swz(Wv[gs, :].T, ET),
                "wpS": _swz(Wp[:, gs].T, CT),
                "bqP": np.ascontiguousarray(bq[gs].reshape(CT, P).T),
                "bkP": np.ascontiguousarray(bk[gs].reshape(CT, P).T),
                "bvB": np.broadcast_to(bv[gs], (P, G)).copy(),
                "maskD": maskDh,
                "maskTxt": maskTx,
            }
        )
    return in_maps


def kernel(x, Wq, bq, Wk, bk, Wv, bv, Wp, bp, seg_starts, seg_ends, T_motion=None,
           N=None, _trace=False, **_unused):
    x = np.asarray(x, np.float32)
    args = [np.asarray(a, np.float32) for a in (Wq, bq, Wk, bk, Wv, bv, Wp, bp)]
    Wq, bq, Wk, bk, Wv, bv, Wp, bp = args
    nc = get_nc()
    in_maps = make_in_maps(x, Wq, bq, Wk, bk, Wv, bv, Wp, bp, seg_starts, seg_ends)
    res = run_bass_kernel_spmd(nc, in_maps, core_ids=list(range(8)), trace=_trace)
    parts = [r["out_part"] for r in res.results]
    y = np.empty((B, L, C), np.float32)
    for b in range(B):
        y[b] = parts[2 * b] + parts[2 * b + 1] + bp
    if _trace:
        kernel.last_results = res
    return y
